# revision 57
# baseline (speedup 1.0000x reference)
"""GAT self-attention Trainium2 kernel (v2: bf16 datapath, overlapped prep).

Full inputs -> shard graphs over 8 NeuronCores -> full output.

Math (per graph n, reference reformulated):
  g_i = sigmoid(relu(q @ W1_i) @ W2_i)            [2d]
  u_i^L = W_i @ (g_i[:d] * a_i[:d])               [k]   (left projector)
  u_i^R = W_i @ (g_i[d:] * a_i[d:])               [k]   (right projector)
  left_i = X @ u_i^L ; right_i = X @ u_i^R        [E]
  score[i,j] = lrelu(left_t[i] + right_t[j]), t = adj[i,j]; -BIG if adj==0
  E = exp(score); rs = rowsum(E); En = E / rs[:,None]
  out = (En^T @ X) @ W_2

Host marshaling: inputs/weights cast to bf16 (adj to int8, lossless); weight
matrices pre-transposed so the device never transposes weights; query vectors
pre-packed in transposed layout. Device does all matmuls/softmax; output is
written fp32 straight from PSUM.
"""
import numpy as np
from contextlib import ExitStack

import concourse.bass as bass
import concourse.tile as tile
from concourse import mybir, bacc
from concourse.masks import make_identity

F32 = mybir.dt.float32
F32R = mybir.dt.float32r
BF16 = mybir.dt.bfloat16
U8 = mybir.dt.uint8
I8 = mybir.dt.int8
AF = mybir.ActivationFunctionType
OP = mybir.AluOpType

N_CORES = 8
N, E, K, D = 64, 512, 512, 512   # graphs, entities, in_dim, out_dim
NG = N // N_CORES                # graphs per core
NT = 3                           # edge types
P = 128
EC = E // P                      # 4 partition chunks of E
KC = K // P
DC2 = (2 * D) // P               # 8 chunks of the 2d gate dim
NEG_BIG = -200.0
LRELU_SLOPE = 0.2
USE_ACT_LRELU = True             # leaky-relu on ACT engine (alpha operand)


def _dma_split(nc, engs, dst, src, pieces):
    """Split a load along dim 1 of dst across the given engine queues."""
    n0 = dst.shape[1]
    step = max(1, n0 // pieces)
    i = 0
    c = 0
    while i < n0:
        j = min(n0, i + step)
        engs[c % len(engs)].dma_start(dst[:, i:j], src[:, i:j])
        i = j
        c += 1


def build(nc, reps=1):
    x = nc.dram_tensor("x", [NG, E, K], BF16, kind="ExternalInput").ap()
    adj = nc.dram_tensor("adj", [NG, E, E], I8, kind="ExternalInput").ap()
    qT = nc.dram_tensor("qT", [P, KC, NG], BF16, kind="ExternalInput").ap()
    aT = nc.dram_tensor("aT", [P, DC2, NT], F32, kind="ExternalInput").ap()
    WtT = nc.dram_tensor("WtT", [NT, D, K], BF16, kind="ExternalInput").ap()
    Wt2 = nc.dram_tensor("Wt2", [K, D], BF16, kind="ExternalInput").ap()
    W1 = nc.dram_tensor("W1", [NT, K, 2 * D], BF16, kind="ExternalInput").ap()
    W2q = nc.dram_tensor("W2q", [NT, 2 * D, 2 * D], BF16, kind="ExternalInput").ap()
    out = nc.dram_tensor("out", [NG, E, D], BF16, kind="ExternalOutput").ap()
    nc._gat_io = (x, adj, qT, aT, WtT, Wt2, W1, W2q, out)
    _build_once(nc, reps)


def _build_once(nc, reps=1):
    x, adj, qT_d, aT_d, WtT, Wt2, W1, W2q, out = nc._gat_io
    with tile.TileContext(nc) as tc, ExitStack() as ctx:
        # ---------------- pools ----------------
        pers = ctx.enter_context(tc.tile_pool(name="pers", bufs=1))
        prep = ctx.enter_context(tc.tile_pool(name="prep", bufs=3))
        deep = ctx.enter_context(tc.tile_pool(name="deep", bufs=3))
        sbuf = ctx.enter_context(tc.tile_pool(name="sbuf", bufs=2))
        small = ctx.enter_context(tc.tile_pool(name="small", bufs=3))
        ps_lr = ctx.enter_context(tc.tile_pool(name="ps_lr", bufs=1, space="PSUM"))
        ps_v = ctx.enter_context(tc.tile_pool(name="ps_v", bufs=4, space="PSUM"))
        ps_big = ctx.enter_context(tc.tile_pool(name="ps_big", bufs=3, space="PSUM"))

        # ---------------- persistent tiles ----------------
        identB = pers.tile([P, P], BF16)
        make_identity(nc, identB[:])
        neg_col = pers.tile([P, 1], F32)
        nc.vector.memset(neg_col[:], NEG_BIG)
        # U_all[k%128, kc, s, i, n]: projectors, order (L1,L2,L3,R1,R2,R3)
        # (bf16: neuronxcc forbids mixing 32-bit with bf16 matmul operands)
        U_all = pers.tile([P, KC, 2, NT, NG], BF16)
        qT_sb = pers.tile([P, KC, NG], BF16)
        aT_sb = pers.tile([P, DC2, NT], F32)
        Wt2_sb = pers.tile([P, KC, D], BF16)

        def phase1(n):
            """weight-light front half: loads, Xt, masks, H = X @ W2"""
            # first two graphs load on the idle sync queue; later prefetches go
            # behind the weight stream on the scalar queue so weights win DMA
            ld = nc.sync if n < 2 else nc.scalar
            adj_sb = deep.tile([P, EC, E], I8, tag="adj")
            ld.dma_start(adj_sb[:], adj[n].rearrange("(c p) j -> p c j", p=P))
            Xt_sb = deep.tile([P, KC, E], BF16, tag="Xt")
            ld.dma_start_transpose(Xt_sb[:], x[n])

            m2 = deep.tile([P, EC, E], U8, tag="m2")
            m3 = deep.tile([P, EC, E], U8, tag="m3")
            nc.gpsimd.tensor_scalar(m2[:], adj_sb[:], 2, None, OP.is_equal)
            nc.gpsimd.tensor_scalar(m3[:], adj_sb[:], 3, None, OP.is_equal)

            # -BIG fill for the score merge, done early off the critical path
            negt = sbuf.tile([P, EC, E], F32, tag="negt")
            nc.gpsimd.memset(negt[:], NEG_BIG)

            # H = X @ W2 (score-independent; keeps PE warm during prep)
            H_sb = deep.tile([P, EC, D], BF16, tag="H")
            for ic in range(EC):
                pH = ps_big.tile([P, D], F32, tag="big")
                for kc in range(KC):
                    nc.tensor.matmul(pH[:], Xt_sb[:, kc, ic * P:(ic + 1) * P],
                                     Wt2_sb[:, kc, :],
                                     start=(kc == 0), stop=(kc == KC - 1))
                nc.scalar.copy(H_sb[:, ic, :], pH[:])
            return dict(H_sb=H_sb, Xt_sb=Xt_sb, adj_sb=adj_sb, m2=m2, m3=m3,
                        negt=negt)

        Wsb = []

        def run_prep_front():
            """loads phase1 of the first graphs depends on"""
            Wsb.clear()
            _dma_split(nc, [nc.scalar], Wt2_sb[:],
                       Wt2.rearrange("(c p) d -> p c d", p=P), 2)
            nc.scalar.dma_start(qT_sb[:], qT_d)
            nc.scalar.dma_start(aT_sb[:], aT_d)

        def run_prep():
            for i in range(NT):
                W1_sb = prep.tile([P, KC, 2 * D], BF16, tag="w1")
                _dma_split(nc, [nc.scalar], W1_sb[:],
                           W1[i].rearrange("(c p) f -> p c f", p=P), 2)
                W2_sb = prep.tile([P, DC2, 2 * D], BF16, tag="w2")
                _dma_split(nc, [nc.scalar], W2_sb[:],
                           W2q[i].rearrange("(c p) f -> p c f", p=P), 4)
                WT_sb = prep.tile([P, EC, K], BF16, tag="wt")
                _dma_split(nc, [nc.scalar], WT_sb[:],
                           WtT[i].rearrange("(c p) k -> p c k", p=P), 2)

                # rr = relu(q @ W1_i):  [NG, 2d] in two 512-halves
                rr_sb = prep.tile([NG, 2 * D], BF16, tag="rr")
                for half in range(2):
                    rp = ps_v.tile([NG, D], F32, tag="v")
                    for kc in range(KC):
                        nc.tensor.matmul(
                            rp[:], qT_sb[:, kc, :],
                            W1_sb[:, kc, half * D:(half + 1) * D],
                            start=(kc == 0), stop=(kc == KC - 1))
                    nc.scalar.activation(rr_sb[:, half * D:(half + 1) * D],
                                         rp[:], AF.Relu)
                # rrT[(2d)%128, dc, n] via PE transposes
                rrT = prep.tile([P, DC2, NG], BF16, tag="rrT")
                trp = ps_big.tile([P, DC2, NG], BF16, tag="big")
                for dc in range(DC2):
                    nc.tensor.transpose(trp[:, dc, :],
                                        rr_sb[:, dc * P:(dc + 1) * P],
                                        identB[:NG, :NG])
                nc.vector.tensor_copy(rrT[:], trp[:])
                # gv = sigmoid(rr @ W2_i)
                gv_sb = prep.tile([NG, 2 * D], BF16, tag="gv")
                for half in range(2):
                    gp = ps_v.tile([NG, D], F32, tag="v")
                    for dc in range(DC2):
                        nc.tensor.matmul(
                            gp[:], rrT[:, dc, :],
                            W2_sb[:, dc, half * D:(half + 1) * D],
                            start=(dc == 0), stop=(dc == DC2 - 1))
                    nc.scalar.activation(gv_sb[:, half * D:(half + 1) * D],
                                         gp[:], AF.Sigmoid)
                # gvT then v = gv * a_i  (broadcast over n)
                trp2 = ps_big.tile([P, DC2, NG], BF16, tag="big")
                for dc in range(DC2):
                    nc.tensor.transpose(trp2[:, dc, :],
                                        gv_sb[:, dc * P:(dc + 1) * P],
                                        identB[:NG, :NG])
                vT = prep.tile([P, DC2, NG], BF16, tag="vT")
                nc.vector.tensor_tensor(
                    vT[:], trp2[:],
                    aT_sb[:, :, i:i + 1].broadcast_to((P, DC2, NG)), OP.mult)
                # U_i(side) = W_i^T-contracted projectors, both sides at once:
                # lhsT = WtT_i chunk [d,128k], rhs = vT[:, {dc, dc+4}, :]
                for kc in range(KC):
                    up = ps_v.tile([P, 2, NG], F32, tag="v")
                    for dc in range(EC):
                        nc.tensor.matmul(
                            up[:], WT_sb[:, dc, kc * P:(kc + 1) * P],
                            vT[:, dc:dc + EC + 1:EC, :],
                            start=(dc == 0), stop=(dc == EC - 1))
                    nc.vector.tensor_copy(U_all[:, kc, :, i, :], up[:])
            # prefill every ring buffer of the LR stacks with ones; per-graph
            # DMAs overwrite only the L/R rows, the ones rows persist
            for _ in range(3):
                Lt = small.tile([66, E], F32R, tag="Lt")
                nc.vector.memset(Lt[:].bitcast(F32), 1.0)
                Rt = small.tile([66, E], F32R, tag="Rt")
                nc.vector.memset(Rt[:].bitcast(F32), 1.0)

        def phase2a(n, st):
            """LR projections + score stacks, run one graph ahead"""
            Xt_sb = st["Xt_sb"]
            pLR = ps_lr.tile([2 * NT, E], F32, tag="lr")
            for kc in range(KC):
                nc.tensor.matmul(pLR[:], U_all[:, kc, :, :, n], Xt_sb[:, kc, :],
                                 start=(kc == 0), stop=(kc == KC - 1))
            # stacks: Lt rows {32t: one, 32t+1: L_t}, Rt rows {32t: R_t, 32t+1: one}.
            # Compute engines can't write strided partitions; stage via SBUF DMA.
            LR_sb = small.tile([2 * NT, E], F32R, tag="LRs")
            nc.scalar.copy(LR_sb[:], pLR[:])
            Lt = small.tile([66, E], F32R, tag="Lt")
            Rt = small.tile([66, E], F32R, tag="Rt")
            nc.sync.dma_start(Lt[1:66:32, :], LR_sb[0:NT, :])
            nc.sync.dma_start(Rt[0:65:32, :], LR_sb[NT:2 * NT, :])
            st["Lt"] = Lt
            st["Rt"] = Rt

        def phase2b(n, st):
            """scores, softmax, out = E^T @ H"""
            H_sb = st["H_sb"]; adj_sb = st["adj_sb"]
            m2 = st["m2"]; m3 = st["m3"]; negt = st["negt"]
            Lt = st["Lt"]; Rt = st["Rt"]

            E_sb = sbuf.tile([P, EC, E], BF16, tag="E")
            rs = small.tile([P, EC], F32, tag="rs")
            rsr = small.tile([P, EC], F32, tag="rsr")
            for ic in range(EC):
                pv = []
                for t in range(NT):
                    pvt = ps_v.tile([P, E], F32, tag="v")
                    nc.tensor.matmul(pvt[:], Lt[32 * t:32 * t + 2, ic * P:(ic + 1) * P],
                                     Rt[32 * t:32 * t + 2, :], start=True, stop=True)
                    pv.append(pvt)
                nc.vector.copy_predicated(pv[0][:], m2[:, ic, :], pv[1][:])
                nc.vector.copy_predicated(pv[0][:], m3[:, ic, :], pv[2][:])
                # adj==0 cells -> NEG_BIG: copy typed cells over a -BIG fill,
                # predicated directly on the raw int8 adj (nonzero = typed)
                nc.vector.copy_predicated(negt[:, ic, :], adj_sb[:, ic, :],
                                          pv[0][:])
                # lrelu in place on SBUF: max(0.2*x, x); -BIG cells stay huge-
                # negative so exp still flushes them to 0. (DVE-only op.)
                nc.vector.scalar_tensor_tensor(negt[:, ic, :], negt[:, ic, :],
                                               LRELU_SLOPE, negt[:, ic, :],
                                               OP.mult, OP.max)
                nc.scalar.activation(E_sb[:, ic, :], negt[:, ic, :], AF.Exp,
                                     accum_out=rs[:, ic:ic + 1])
                nc.vector.reciprocal(rsr[:, ic:ic + 1], rs[:, ic:ic + 1])
                nc.vector.tensor_scalar(E_sb[:, ic, :], E_sb[:, ic, :],
                                        rsr[:, ic:ic + 1], None, OP.mult)
            st["E_sb"] = E_sb

        def phase2c(n, st):
            """out = E^T @ H (emitted after the next phase1 so its H matmuls
            can fill the PE gap while this graph's softmax finishes)"""
            E_sb = st["E_sb"]; H_sb = st["H_sb"]
            for jc in range(EC):
                pO = ps_big.tile([P, D], F32, tag="big")
                for ic in range(EC):
                    nc.tensor.matmul(pO[:], E_sb[:, ic, jc * P:(jc + 1) * P],
                                     H_sb[:, ic, :],
                                     start=(ic == 0), stop=(ic == EC - 1))
                o_sb = small.tile([P, D], BF16, tag="osb")
                nc.scalar.copy(o_sb[:], pO[:])
                nc.sync.dma_start(out[n, jc * P:(jc + 1) * P, :], o_sb[:])

        def body_all(_iv=None):
            sts = {}
            run_prep_front()
            sts[0] = phase1(0)
            sts[1] = phase1(1)
            run_prep()
            phase2a(0, sts[0])
            for n in range(NG):
                if n + 1 < NG:
                    phase2a(n + 1, sts[n + 1])
                st = sts.pop(n)
                phase2b(n, st)
                phase2c(n, st)
                if n + 2 < NG:
                    sts[n + 2] = phase1(n + 2)

        if reps == 1:
            body_all()
        else:
            with tc.For_i(0, reps, 1) as _iv:
                body_all(_iv)
    return nc


_NC_CACHE = {}
TRACE = False
_LAST = {}


def _get_nc():
    if "nc" not in _NC_CACHE:
        nc = bacc.Bacc("TRN2", target_bir_lowering=False, debug=False)
        build(nc)
        nc.compile()
        _NC_CACHE["nc"] = nc
    return _NC_CACHE["nc"]


def kernel(input_state, adj, entity_mask, query_vec, W_type, a_type,
           qattn_W1, qattn_W2):
    import ml_dtypes
    from concourse import bass_utils
    bf16 = ml_dtypes.bfloat16
    nc = _get_nc()

    x_bf = np.ascontiguousarray(input_state, dtype=np.float32).astype(bf16)
    adj_i8 = np.ascontiguousarray(adj).astype(np.int8)
    qv = np.ascontiguousarray(query_vec, dtype=np.float32).astype(bf16)
    # aT[p, dc2, t] = a_type[t, dc2*128 + p]
    aT = np.ascontiguousarray(
        np.transpose(np.asarray(a_type, np.float32).reshape(NT, DC2, P),
                     (2, 1, 0)))
    WtT = np.ascontiguousarray(
        np.transpose(np.asarray(W_type, np.float32), (0, 2, 1))).astype(bf16)
    Wt2 = np.ascontiguousarray(np.asarray(W_type, np.float32)[2]).astype(bf16)
    W1_bf = np.ascontiguousarray(qattn_W1, dtype=np.float32).astype(bf16)
    W2_bf = np.ascontiguousarray(qattn_W2, dtype=np.float32).astype(bf16)

    in_maps = []
    for c in range(N_CORES):
        sl = slice(c * NG, (c + 1) * NG)
        # qT[p, kc, n] = qv[n, kc*128 + p]
        qT = np.ascontiguousarray(
            np.transpose(qv[sl].reshape(NG, KC, P), (2, 1, 0)))
        in_maps.append({
            "x": x_bf[sl], "adj": adj_i8[sl], "qT": qT, "aT": aT,
            "WtT": WtT, "Wt2": Wt2, "W1": W1_bf, "W2q": W2_bf,
        })
    res = bass_utils.run_bass_kernel_spmd(nc, in_maps, core_ids=list(range(N_CORES)),
                                          trace=TRACE, stitch_traces=TRACE)
    _LAST["exec_ns"] = res.exec_time_ns
    _LAST["mean_ns"] = res.mean_exec_time_ns
    _LAST["trace"] = res.instructions_and_trace
    _LAST["scope_times"] = res.per_core_scope_times
    out = np.concatenate([np.asarray(r["out"], np.float32) for r in res.results],
                         axis=0)
    return out


# revision 58
# speedup vs baseline: 1.0731x; 1.0731x over previous
"""GAT self-attention Trainium2 kernel (v2: bf16 datapath, overlapped prep).

Full inputs -> shard graphs over 8 NeuronCores -> full output.

Math (per graph n, reference reformulated):
  g_i = sigmoid(relu(q @ W1_i) @ W2_i)            [2d]
  u_i^L = W_i @ (g_i[:d] * a_i[:d])               [k]   (left projector)
  u_i^R = W_i @ (g_i[d:] * a_i[d:])               [k]   (right projector)
  left_i = X @ u_i^L ; right_i = X @ u_i^R        [E]
  score[i,j] = lrelu(left_t[i] + right_t[j]), t = adj[i,j]; -BIG if adj==0
  E = exp(score); rs = rowsum(E); En = E / rs[:,None]
  out = (En^T @ X) @ W_2

Host marshaling: inputs/weights cast to bf16 (adj to int8, lossless); weight
matrices pre-transposed so the device never transposes weights; query vectors
pre-packed in transposed layout. Device does all matmuls/softmax; output is
written fp32 straight from PSUM.
"""
import numpy as np
from contextlib import ExitStack

import concourse.bass as bass
import concourse.tile as tile
from concourse import mybir, bacc
from concourse.masks import make_identity

F32 = mybir.dt.float32
F32R = mybir.dt.float32r
BF16 = mybir.dt.bfloat16
U8 = mybir.dt.uint8
I8 = mybir.dt.int8
AF = mybir.ActivationFunctionType
OP = mybir.AluOpType

N_CORES = 8
N, E, K, D = 64, 512, 512, 512   # graphs, entities, in_dim, out_dim
NG = N // N_CORES                # graphs per core
NT = 3                           # edge types
P = 128
EC = E // P                      # 4 partition chunks of E
KC = K // P
DC2 = (2 * D) // P               # 8 chunks of the 2d gate dim
NEG_BIG = -200.0
LRELU_SLOPE = 0.2
USE_ACT_LRELU = True             # leaky-relu on ACT engine (alpha operand)


def _dma_split(nc, engs, dst, src, pieces):
    """Split a load along dim 1 of dst across the given engine queues."""
    n0 = dst.shape[1]
    step = max(1, n0 // pieces)
    i = 0
    c = 0
    while i < n0:
        j = min(n0, i + step)
        engs[c % len(engs)].dma_start(dst[:, i:j], src[:, i:j])
        i = j
        c += 1


def build(nc, reps=1):
    x = nc.dram_tensor("x", [NG, E, K], BF16, kind="ExternalInput").ap()
    adj = nc.dram_tensor("adj", [NG, E, E], I8, kind="ExternalInput").ap()
    qT = nc.dram_tensor("qT", [P, KC, NG], BF16, kind="ExternalInput").ap()
    aT = nc.dram_tensor("aT", [P, DC2, NT], F32, kind="ExternalInput").ap()
    WtT = nc.dram_tensor("WtT", [NT, D, K], BF16, kind="ExternalInput").ap()
    Wt2 = nc.dram_tensor("Wt2", [K, D], BF16, kind="ExternalInput").ap()
    W1 = nc.dram_tensor("W1", [NT, K, 2 * D], BF16, kind="ExternalInput").ap()
    W2q = nc.dram_tensor("W2q", [NT, 2 * D, 2 * D], BF16, kind="ExternalInput").ap()
    out = nc.dram_tensor("out", [NG, E, D], BF16, kind="ExternalOutput").ap()
    nc._gat_io = (x, adj, qT, aT, WtT, Wt2, W1, W2q, out)
    _build_once(nc, reps)


def _build_once(nc, reps=1):
    x, adj, qT_d, aT_d, WtT, Wt2, W1, W2q, out = nc._gat_io
    with tile.TileContext(nc) as tc, ExitStack() as ctx:
        # ---------------- pools ----------------
        pers = ctx.enter_context(tc.tile_pool(name="pers", bufs=1))
        prep = ctx.enter_context(tc.tile_pool(name="prep", bufs=3))
        deep = ctx.enter_context(tc.tile_pool(name="deep", bufs=3))
        sbuf = ctx.enter_context(tc.tile_pool(name="sbuf", bufs=2))
        small = ctx.enter_context(tc.tile_pool(name="small", bufs=3))
        ps_lr = ctx.enter_context(tc.tile_pool(name="ps_lr", bufs=1, space="PSUM"))
        ps_v = ctx.enter_context(tc.tile_pool(name="ps_v", bufs=5, space="PSUM"))
        ps_big = ctx.enter_context(tc.tile_pool(name="ps_big", bufs=2, space="PSUM"))

        # ---------------- persistent tiles ----------------
        identB = pers.tile([P, P], BF16)
        make_identity(nc, identB[:])
        neg_col = pers.tile([P, 1], F32)
        nc.vector.memset(neg_col[:], NEG_BIG)
        # U_all[k%128, kc, s, i, n]: projectors, order (L1,L2,L3,R1,R2,R3)
        # (bf16: neuronxcc forbids mixing 32-bit with bf16 matmul operands)
        U_all = pers.tile([P, KC, 2, NT, NG], BF16)
        qT_sb = pers.tile([P, KC, NG], BF16)
        aT_sb = pers.tile([P, DC2, NT], F32)
        Wt2_sb = pers.tile([P, KC, D], BF16)

        def phase1(n):
            """weight-light front half: loads, Xt, masks, H = X @ W2"""
            # first two graphs load on the idle sync queue; later prefetches go
            # behind the weight stream on the scalar queue so weights win DMA
            ld = nc.sync if n < 2 else nc.scalar
            adj_sb = deep.tile([P, EC, E], I8, tag="adj")
            ld.dma_start(adj_sb[:], adj[n].rearrange("(c p) j -> p c j", p=P))
            Xt_sb = deep.tile([P, KC, E], BF16, tag="Xt")
            ld.dma_start_transpose(Xt_sb[:], x[n])

            m2 = deep.tile([P, EC, E], U8, tag="m2")
            m3 = deep.tile([P, EC, E], U8, tag="m3")
            nc.gpsimd.tensor_scalar(m2[:], adj_sb[:], 2, None, OP.is_equal)
            nc.gpsimd.tensor_scalar(m3[:], adj_sb[:], 3, None, OP.is_equal)

            # -BIG fill for the score merge, done early off the critical path
            negt = sbuf.tile([P, EC, E], F32, tag="negt")
            nc.gpsimd.memset(negt[:], NEG_BIG)

            # H = X @ W2 (score-independent; keeps PE warm during prep)
            H_sb = deep.tile([P, EC, D], BF16, tag="H")
            for ic in range(EC):
                pH = ps_big.tile([P, D], F32, tag="big")
                for kc in range(KC):
                    nc.tensor.matmul(pH[:], Xt_sb[:, kc, ic * P:(ic + 1) * P],
                                     Wt2_sb[:, kc, :],
                                     start=(kc == 0), stop=(kc == KC - 1))
                nc.scalar.copy(H_sb[:, ic, :], pH[:])
            return dict(H_sb=H_sb, Xt_sb=Xt_sb, adj_sb=adj_sb, m2=m2, m3=m3,
                        negt=negt)

        Wsb = []

        def run_prep_front():
            """loads phase1 of the first graphs depends on"""
            Wsb.clear()
            _dma_split(nc, [nc.scalar], Wt2_sb[:],
                       Wt2.rearrange("(c p) d -> p c d", p=P), 2)
            nc.scalar.dma_start(qT_sb[:], qT_d)
            nc.scalar.dma_start(aT_sb[:], aT_d)

        def run_prep():
            for i in range(NT):
                W1_sb = prep.tile([P, KC, 2 * D], BF16, tag="w1")
                _dma_split(nc, [nc.scalar], W1_sb[:],
                           W1[i].rearrange("(c p) f -> p c f", p=P), 2)
                W2_sb = prep.tile([P, DC2, 2 * D], BF16, tag="w2")
                _dma_split(nc, [nc.scalar], W2_sb[:],
                           W2q[i].rearrange("(c p) f -> p c f", p=P), 4)
                WT_sb = prep.tile([P, EC, K], BF16, tag="wt")
                _dma_split(nc, [nc.scalar], WT_sb[:],
                           WtT[i].rearrange("(c p) k -> p c k", p=P), 2)

                # rr = relu(q @ W1_i):  [NG, 2d] in two 512-halves
                rr_sb = prep.tile([NG, 2 * D], BF16, tag="rr")
                for half in range(2):
                    rp = ps_v.tile([NG, D], F32, tag="v")
                    for kc in range(KC):
                        nc.tensor.matmul(
                            rp[:], qT_sb[:, kc, :],
                            W1_sb[:, kc, half * D:(half + 1) * D],
                            start=(kc == 0), stop=(kc == KC - 1))
                    nc.scalar.activation(rr_sb[:, half * D:(half + 1) * D],
                                         rp[:], AF.Relu)
                # rrT[(2d)%128, dc, n] via PE transposes
                rrT = prep.tile([P, DC2, NG], BF16, tag="rrT")
                trp = ps_big.tile([P, DC2, NG], BF16, tag="big")
                for dc in range(DC2):
                    nc.tensor.transpose(trp[:, dc, :],
                                        rr_sb[:, dc * P:(dc + 1) * P],
                                        identB[:NG, :NG])
                nc.vector.tensor_copy(rrT[:], trp[:])
                # gv = sigmoid(rr @ W2_i)
                gv_sb = prep.tile([NG, 2 * D], BF16, tag="gv")
                for half in range(2):
                    gp = ps_v.tile([NG, D], F32, tag="v")
                    for dc in range(DC2):
                        nc.tensor.matmul(
                            gp[:], rrT[:, dc, :],
                            W2_sb[:, dc, half * D:(half + 1) * D],
                            start=(dc == 0), stop=(dc == DC2 - 1))
                    nc.scalar.activation(gv_sb[:, half * D:(half + 1) * D],
                                         gp[:], AF.Sigmoid)
                # gvT then v = gv * a_i  (broadcast over n)
                trp2 = ps_big.tile([P, DC2, NG], BF16, tag="big")
                for dc in range(DC2):
                    nc.tensor.transpose(trp2[:, dc, :],
                                        gv_sb[:, dc * P:(dc + 1) * P],
                                        identB[:NG, :NG])
                vT = prep.tile([P, DC2, NG], BF16, tag="vT")
                nc.vector.tensor_tensor(
                    vT[:], trp2[:],
                    aT_sb[:, :, i:i + 1].broadcast_to((P, DC2, NG)), OP.mult)
                # U_i(side) = W_i^T-contracted projectors, both sides at once:
                # lhsT = WtT_i chunk [d,128k], rhs = vT[:, {dc, dc+4}, :]
                for kc in range(KC):
                    up = ps_v.tile([P, 2, NG], F32, tag="v")
                    for dc in range(EC):
                        nc.tensor.matmul(
                            up[:], WT_sb[:, dc, kc * P:(kc + 1) * P],
                            vT[:, dc:dc + EC + 1:EC, :],
                            start=(dc == 0), stop=(dc == EC - 1))
                    nc.vector.tensor_copy(U_all[:, kc, :, i, :], up[:])
            # prefill every ring buffer of the LR stacks with ones; per-graph
            # DMAs overwrite only the L/R rows, the ones rows persist
            for _ in range(3):
                Lt = small.tile([66, E], F32R, tag="Lt")
                nc.vector.memset(Lt[:].bitcast(F32), 1.0)
                Rt = small.tile([66, E], F32R, tag="Rt")
                nc.vector.memset(Rt[:].bitcast(F32), 1.0)

        def phase2a(n, st):
            """LR projections + score stacks, run one graph ahead"""
            Xt_sb = st["Xt_sb"]
            pLR = ps_lr.tile([2 * NT, E], F32, tag="lr")
            for kc in range(KC):
                nc.tensor.matmul(pLR[:], U_all[:, kc, :, :, n], Xt_sb[:, kc, :],
                                 start=(kc == 0), stop=(kc == KC - 1))
            # stacks: Lt rows {32t: one, 32t+1: L_t}, Rt rows {32t: R_t, 32t+1: one}.
            # Compute engines can't write strided partitions; stage via SBUF DMA.
            LR_sb = small.tile([2 * NT, E], F32R, tag="LRs")
            nc.scalar.copy(LR_sb[:], pLR[:])
            Lt = small.tile([66, E], F32R, tag="Lt")
            Rt = small.tile([66, E], F32R, tag="Rt")
            nc.sync.dma_start(Lt[1:66:32, :], LR_sb[0:NT, :])
            nc.sync.dma_start(Rt[0:65:32, :], LR_sb[NT:2 * NT, :])
            st["Lt"] = Lt
            st["Rt"] = Rt

        def phase2b(n, st):
            """scores, softmax, out = E^T @ H"""
            H_sb = st["H_sb"]; adj_sb = st["adj_sb"]
            m2 = st["m2"]; m3 = st["m3"]; negt = st["negt"]
            Lt = st["Lt"]; Rt = st["Rt"]

            E_sb = sbuf.tile([P, EC, E], BF16, tag="E")
            rs = small.tile([P, EC], F32, tag="rs")
            rsr = small.tile([P, EC], F32, tag="rsr")
            for ic in range(EC):
                pv = []
                for t in range(NT):
                    pvt = ps_v.tile([P, E], F32, tag="v")
                    nc.tensor.matmul(pvt[:], Lt[32 * t:32 * t + 2, ic * P:(ic + 1) * P],
                                     Rt[32 * t:32 * t + 2, :], start=True, stop=True)
                    pv.append(pvt)
                nc.vector.copy_predicated(pv[0][:], m2[:, ic, :], pv[1][:])
                nc.vector.copy_predicated(pv[0][:], m3[:, ic, :], pv[2][:])
                # adj==0 cells -> NEG_BIG: copy typed cells over a -BIG fill,
                # predicated directly on the raw int8 adj (nonzero = typed)
                nc.vector.copy_predicated(negt[:, ic, :], adj_sb[:, ic, :],
                                          pv[0][:])
                # lrelu in place on SBUF: max(0.2*x, x); -BIG cells stay huge-
                # negative so exp still flushes them to 0. (DVE-only op.)
                nc.vector.scalar_tensor_tensor(negt[:, ic, :], negt[:, ic, :],
                                               LRELU_SLOPE, negt[:, ic, :],
                                               OP.mult, OP.max)
                nc.scalar.activation(E_sb[:, ic, :], negt[:, ic, :], AF.Exp,
                                     accum_out=rs[:, ic:ic + 1])
                nc.vector.reciprocal(rsr[:, ic:ic + 1], rs[:, ic:ic + 1])
                nc.vector.tensor_scalar(E_sb[:, ic, :], E_sb[:, ic, :],
                                        rsr[:, ic:ic + 1], None, OP.mult)
            st["E_sb"] = E_sb

        def phase2c(n, st):
            """out = E^T @ H (emitted after the next phase1 so its H matmuls
            can fill the PE gap while this graph's softmax finishes)"""
            E_sb = st["E_sb"]; H_sb = st["H_sb"]
            for jc in range(EC):
                pO = ps_big.tile([P, D], F32, tag="big")
                for ic in range(EC):
                    nc.tensor.matmul(pO[:], E_sb[:, ic, jc * P:(jc + 1) * P],
                                     H_sb[:, ic, :],
                                     start=(ic == 0), stop=(ic == EC - 1))
                o_sb = small.tile([P, D], BF16, tag="osb")
                nc.scalar.copy(o_sb[:], pO[:])
                nc.sync.dma_start(out[n, jc * P:(jc + 1) * P, :], o_sb[:])

        def body_all(_iv=None):
            sts = {}
            run_prep_front()
            sts[0] = phase1(0)
            sts[1] = phase1(1)
            run_prep()
            phase2a(0, sts[0])
            for n in range(NG):
                if n + 1 < NG:
                    phase2a(n + 1, sts[n + 1])
                st = sts.pop(n)
                phase2b(n, st)
                phase2c(n, st)
                if n + 2 < NG:
                    sts[n + 2] = phase1(n + 2)

        if reps == 1:
            body_all()
        else:
            with tc.For_i(0, reps, 1) as _iv:
                body_all(_iv)
    return nc


_NC_CACHE = {}
TRACE = False
_LAST = {}


def _get_nc():
    if "nc" not in _NC_CACHE:
        nc = bacc.Bacc("TRN2", target_bir_lowering=False, debug=False)
        build(nc)
        nc.compile()
        _NC_CACHE["nc"] = nc
    return _NC_CACHE["nc"]


def kernel(input_state, adj, entity_mask, query_vec, W_type, a_type,
           qattn_W1, qattn_W2):
    import ml_dtypes
    from concourse import bass_utils
    bf16 = ml_dtypes.bfloat16
    nc = _get_nc()

    x_bf = np.ascontiguousarray(input_state, dtype=np.float32).astype(bf16)
    adj_i8 = np.ascontiguousarray(adj).astype(np.int8)
    qv = np.ascontiguousarray(query_vec, dtype=np.float32).astype(bf16)
    # aT[p, dc2, t] = a_type[t, dc2*128 + p]
    aT = np.ascontiguousarray(
        np.transpose(np.asarray(a_type, np.float32).reshape(NT, DC2, P),
                     (2, 1, 0)))
    WtT = np.ascontiguousarray(
        np.transpose(np.asarray(W_type, np.float32), (0, 2, 1))).astype(bf16)
    Wt2 = np.ascontiguousarray(np.asarray(W_type, np.float32)[2]).astype(bf16)
    W1_bf = np.ascontiguousarray(qattn_W1, dtype=np.float32).astype(bf16)
    W2_bf = np.ascontiguousarray(qattn_W2, dtype=np.float32).astype(bf16)

    in_maps = []
    for c in range(N_CORES):
        sl = slice(c * NG, (c + 1) * NG)
        # qT[p, kc, n] = qv[n, kc*128 + p]
        qT = np.ascontiguousarray(
            np.transpose(qv[sl].reshape(NG, KC, P), (2, 1, 0)))
        in_maps.append({
            "x": x_bf[sl], "adj": adj_i8[sl], "qT": qT, "aT": aT,
            "WtT": WtT, "Wt2": Wt2, "W1": W1_bf, "W2q": W2_bf,
        })
    res = bass_utils.run_bass_kernel_spmd(nc, in_maps, core_ids=list(range(N_CORES)),
                                          trace=TRACE, stitch_traces=TRACE)
    _LAST["exec_ns"] = res.exec_time_ns
    _LAST["mean_ns"] = res.mean_exec_time_ns
    _LAST["trace"] = res.instructions_and_trace
    _LAST["scope_times"] = res.per_core_scope_times
    out = np.concatenate([np.asarray(r["out"], np.float32) for r in res.results],
                         axis=0)
    return out


# revision 59
# speedup vs baseline: 1.0866x; 1.0126x over previous
"""GAT self-attention Trainium2 kernel (v2: bf16 datapath, overlapped prep).

Full inputs -> shard graphs over 8 NeuronCores -> full output.

Math (per graph n, reference reformulated):
  g_i = sigmoid(relu(q @ W1_i) @ W2_i)            [2d]
  u_i^L = W_i @ (g_i[:d] * a_i[:d])               [k]   (left projector)
  u_i^R = W_i @ (g_i[d:] * a_i[d:])               [k]   (right projector)
  left_i = X @ u_i^L ; right_i = X @ u_i^R        [E]
  score[i,j] = lrelu(left_t[i] + right_t[j]), t = adj[i,j]; -BIG if adj==0
  E = exp(score); rs = rowsum(E); En = E / rs[:,None]
  out = (En^T @ X) @ W_2

Host marshaling: inputs/weights cast to bf16 (adj to int8, lossless); weight
matrices pre-transposed so the device never transposes weights; query vectors
pre-packed in transposed layout. Device does all matmuls/softmax; output is
written fp32 straight from PSUM.
"""
import numpy as np
from contextlib import ExitStack

import concourse.bass as bass
import concourse.tile as tile
from concourse import mybir, bacc
from concourse.masks import make_identity

F32 = mybir.dt.float32
F32R = mybir.dt.float32r
BF16 = mybir.dt.bfloat16
U8 = mybir.dt.uint8
I8 = mybir.dt.int8
AF = mybir.ActivationFunctionType
OP = mybir.AluOpType

N_CORES = 8
N, E, K, D = 64, 512, 512, 512   # graphs, entities, in_dim, out_dim
NG = N // N_CORES                # graphs per core
NT = 3                           # edge types
P = 128
EC = E // P                      # 4 partition chunks of E
KC = K // P
DC2 = (2 * D) // P               # 8 chunks of the 2d gate dim
NEG_BIG = -200.0
LRELU_SLOPE = 0.2
USE_ACT_LRELU = True             # leaky-relu on ACT engine (alpha operand)


def _dma_split(nc, engs, dst, src, pieces):
    """Split a load along dim 1 of dst across the given engine queues."""
    n0 = dst.shape[1]
    step = max(1, n0 // pieces)
    i = 0
    c = 0
    while i < n0:
        j = min(n0, i + step)
        engs[c % len(engs)].dma_start(dst[:, i:j], src[:, i:j])
        i = j
        c += 1


def build(nc, reps=1):
    x = nc.dram_tensor("x", [NG, E, K], BF16, kind="ExternalInput").ap()
    adj = nc.dram_tensor("adj", [NG, E, E], I8, kind="ExternalInput").ap()
    qT = nc.dram_tensor("qT", [P, KC, NG], BF16, kind="ExternalInput").ap()
    aT = nc.dram_tensor("aT", [P, DC2, NT], F32, kind="ExternalInput").ap()
    WtT = nc.dram_tensor("WtT", [NT, D, K], BF16, kind="ExternalInput").ap()
    Wt2 = nc.dram_tensor("Wt2", [K, D], BF16, kind="ExternalInput").ap()
    W1 = nc.dram_tensor("W1", [NT, K, 2 * D], BF16, kind="ExternalInput").ap()
    W2q = nc.dram_tensor("W2q", [NT, 2 * D, 2 * D], BF16, kind="ExternalInput").ap()
    out = nc.dram_tensor("out", [NG, E, D], BF16, kind="ExternalOutput").ap()
    nc._gat_io = (x, adj, qT, aT, WtT, Wt2, W1, W2q, out)
    _build_once(nc, reps)


def _build_once(nc, reps=1):
    x, adj, qT_d, aT_d, WtT, Wt2, W1, W2q, out = nc._gat_io
    with tile.TileContext(nc) as tc, ExitStack() as ctx:
        # ---------------- pools ----------------
        pers = ctx.enter_context(tc.tile_pool(name="pers", bufs=1))
        prep = ctx.enter_context(tc.tile_pool(name="prep", bufs=3))
        deep = ctx.enter_context(tc.tile_pool(name="deep", bufs=3))
        sbuf = ctx.enter_context(tc.tile_pool(name="sbuf", bufs=2))
        small = ctx.enter_context(tc.tile_pool(name="small", bufs=3))
        ps_lr = ctx.enter_context(tc.tile_pool(name="ps_lr", bufs=1, space="PSUM"))
        ps_v = ctx.enter_context(tc.tile_pool(name="ps_v", bufs=5, space="PSUM"))
        ps_big = ctx.enter_context(tc.tile_pool(name="ps_big", bufs=2, space="PSUM"))

        # ---------------- persistent tiles ----------------
        identB = pers.tile([P, P], BF16)
        make_identity(nc, identB[:])
        neg_col = pers.tile([P, 1], F32)
        nc.vector.memset(neg_col[:], NEG_BIG)
        # U_all[k%128, kc, s, i, n]: projectors, order (L1,L2,L3,R1,R2,R3)
        # (bf16: neuronxcc forbids mixing 32-bit with bf16 matmul operands)
        U_all = pers.tile([P, KC, 2, NT, NG], BF16)
        qT_sb = pers.tile([P, KC, NG], BF16)
        aT_sb = pers.tile([P, DC2, NT], F32)
        Wt2_sb = pers.tile([P, KC, D], BF16)

        def phase1(n):
            """weight-light front half: loads, Xt, masks, H = X @ W2"""
            # first two graphs load on the idle sync queue; later prefetches go
            # behind the weight stream on the scalar queue so weights win DMA
            ld = nc.sync if n < 2 else nc.scalar
            adj_sb = deep.tile([P, EC, E], I8, tag="adj")
            ld.dma_start(adj_sb[:], adj[n].rearrange("(c p) j -> p c j", p=P))
            Xt_sb = deep.tile([P, KC, E], BF16, tag="Xt")
            ld.dma_start_transpose(Xt_sb[:], x[n])

            m2 = deep.tile([P, EC, E], U8, tag="m2")
            m3 = deep.tile([P, EC, E], U8, tag="m3")
            nc.gpsimd.tensor_scalar(m2[:], adj_sb[:], 2, None, OP.is_equal)
            nc.gpsimd.tensor_scalar(m3[:], adj_sb[:], 3, None, OP.is_equal)

            # -BIG fill for the score merge, done early off the critical path
            negt = sbuf.tile([P, EC, E], F32, tag="negt")
            nc.gpsimd.memset(negt[:], NEG_BIG)

            # H = X @ W2 (score-independent; keeps PE warm during prep)
            H_sb = deep.tile([P, EC, D], BF16, tag="H")
            for ic in range(EC):
                pH = ps_big.tile([P, D], F32, tag="big")
                for kc in range(KC):
                    nc.tensor.matmul(pH[:], Xt_sb[:, kc, ic * P:(ic + 1) * P],
                                     Wt2_sb[:, kc, :],
                                     start=(kc == 0), stop=(kc == KC - 1))
                nc.scalar.copy(H_sb[:, ic, :], pH[:])
            return dict(H_sb=H_sb, Xt_sb=Xt_sb, adj_sb=adj_sb, m2=m2, m3=m3,
                        negt=negt)

        Wsb = []

        def run_prep_front():
            """loads phase1 of the first graphs depends on"""
            Wsb.clear()
            _dma_split(nc, [nc.scalar], Wt2_sb[:],
                       Wt2.rearrange("(c p) d -> p c d", p=P), 2)
            nc.scalar.dma_start(qT_sb[:], qT_d)
            nc.scalar.dma_start(aT_sb[:], aT_d)

        def run_prep():
            for i in range(NT):
                W1_sb = prep.tile([P, KC, 2 * D], BF16, tag="w1")
                _dma_split(nc, [nc.scalar], W1_sb[:],
                           W1[i].rearrange("(c p) f -> p c f", p=P), 2)
                W2_sb = prep.tile([P, DC2, 2 * D], BF16, tag="w2")
                _dma_split(nc, [nc.scalar], W2_sb[:],
                           W2q[i].rearrange("(c p) f -> p c f", p=P), 4)
                WT_sb = prep.tile([P, EC, K], BF16, tag="wt")
                _dma_split(nc, [nc.scalar], WT_sb[:],
                           WtT[i].rearrange("(c p) k -> p c k", p=P), 2)

                # rr = relu(q @ W1_i):  [NG, 2d] in two 512-halves
                rr_sb = prep.tile([NG, 2 * D], BF16, tag="rr")
                for half in range(2):
                    rp = ps_v.tile([NG, D], F32, tag="v")
                    for kc in range(KC):
                        nc.tensor.matmul(
                            rp[:], qT_sb[:, kc, :],
                            W1_sb[:, kc, half * D:(half + 1) * D],
                            start=(kc == 0), stop=(kc == KC - 1))
                    nc.scalar.activation(rr_sb[:, half * D:(half + 1) * D],
                                         rp[:], AF.Relu)
                # rrT[(2d)%128, dc, n] via PE transposes
                rrT = prep.tile([P, DC2, NG], BF16, tag="rrT")
                trp = ps_big.tile([P, DC2, NG], BF16, tag="big")
                for dc in range(DC2):
                    nc.tensor.transpose(trp[:, dc, :],
                                        rr_sb[:, dc * P:(dc + 1) * P],
                                        identB[:NG, :NG])
                nc.vector.tensor_copy(rrT[:], trp[:])
                # gv = sigmoid(rr @ W2_i)
                gv_sb = prep.tile([NG, 2 * D], BF16, tag="gv")
                for half in range(2):
                    gp = ps_v.tile([NG, D], F32, tag="v")
                    for dc in range(DC2):
                        nc.tensor.matmul(
                            gp[:], rrT[:, dc, :],
                            W2_sb[:, dc, half * D:(half + 1) * D],
                            start=(dc == 0), stop=(dc == DC2 - 1))
                    nc.scalar.activation(gv_sb[:, half * D:(half + 1) * D],
                                         gp[:], AF.Sigmoid)
                # gvT then v = gv * a_i  (broadcast over n)
                trp2 = ps_big.tile([P, DC2, NG], BF16, tag="big")
                for dc in range(DC2):
                    nc.tensor.transpose(trp2[:, dc, :],
                                        gv_sb[:, dc * P:(dc + 1) * P],
                                        identB[:NG, :NG])
                vT = prep.tile([P, DC2, NG], BF16, tag="vT")
                nc.vector.tensor_tensor(
                    vT[:], trp2[:],
                    aT_sb[:, :, i:i + 1].broadcast_to((P, DC2, NG)), OP.mult)
                # U_i(side) = W_i^T-contracted projectors, both sides at once:
                # lhsT = WtT_i chunk [d,128k], rhs = vT[:, {dc, dc+4}, :]
                for kc in range(KC):
                    up = ps_v.tile([P, 2, NG], F32, tag="v")
                    for dc in range(EC):
                        nc.tensor.matmul(
                            up[:], WT_sb[:, dc, kc * P:(kc + 1) * P],
                            vT[:, dc:dc + EC + 1:EC, :],
                            start=(dc == 0), stop=(dc == EC - 1))
                    nc.vector.tensor_copy(U_all[:, kc, :, i, :], up[:])
            # prefill every ring buffer of the LR stacks with ones; per-graph
            # DMAs overwrite only the L/R rows, the ones rows persist
            for _ in range(3):
                Lt = small.tile([66, E], F32R, tag="Lt")
                nc.vector.memset(Lt[:].bitcast(F32), 1.0)
                Rt = small.tile([66, E], F32R, tag="Rt")
                nc.vector.memset(Rt[:].bitcast(F32), 1.0)

        def phase2a(n, st):
            """LR projections + score stacks, run one graph ahead"""
            Xt_sb = st["Xt_sb"]
            pLR = ps_lr.tile([2 * NT, E], F32, tag="lr")
            for kc in range(KC):
                nc.tensor.matmul(pLR[:], U_all[:, kc, :, :, n], Xt_sb[:, kc, :],
                                 start=(kc == 0), stop=(kc == KC - 1))
            # stacks: Lt rows {32t: one, 32t+1: L_t}, Rt rows {32t: R_t, 32t+1: one}.
            # Compute engines can't write strided partitions; stage via SBUF DMA.
            LR_sb = small.tile([2 * NT, E], F32R, tag="LRs")
            nc.scalar.copy(LR_sb[:], pLR[:])
            Lt = small.tile([66, E], F32R, tag="Lt")
            Rt = small.tile([66, E], F32R, tag="Rt")
            nc.sync.dma_start(Lt[1:66:32, :], LR_sb[0:NT, :])
            nc.sync.dma_start(Rt[0:65:32, :], LR_sb[NT:2 * NT, :])
            st["Lt"] = Lt
            st["Rt"] = Rt

        def phase2b(n, st):
            """scores, softmax, out = E^T @ H"""
            H_sb = st["H_sb"]; adj_sb = st["adj_sb"]
            m2 = st["m2"]; m3 = st["m3"]; negt = st["negt"]
            Lt = st["Lt"]; Rt = st["Rt"]

            E_sb = sbuf.tile([P, EC, E], BF16, tag="E")
            rs = small.tile([P, EC], F32, tag="rs")
            rsr = small.tile([P, EC], F32, tag="rsr")
            for ic in range(EC):
                pv = []
                for t in range(NT):
                    pvt = ps_v.tile([P, E], F32, tag="v")
                    nc.tensor.matmul(pvt[:], Lt[32 * t:32 * t + 2, ic * P:(ic + 1) * P],
                                     Rt[32 * t:32 * t + 2, :], start=True, stop=True)
                    pv.append(pvt)
                nc.vector.copy_predicated(pv[0][:], m2[:, ic, :], pv[1][:])
                nc.vector.copy_predicated(pv[0][:], m3[:, ic, :], pv[2][:])
                # adj==0 cells -> NEG_BIG: copy typed cells over a -BIG fill,
                # predicated directly on the raw int8 adj (nonzero = typed)
                nc.vector.copy_predicated(negt[:, ic, :], adj_sb[:, ic, :],
                                          pv[0][:])
                # lrelu in place on SBUF: max(0.2*x, x); -BIG cells stay huge-
                # negative so exp still flushes them to 0. (DVE-only op.)
                nc.vector.scalar_tensor_tensor(negt[:, ic, :], negt[:, ic, :],
                                               LRELU_SLOPE, negt[:, ic, :],
                                               OP.mult, OP.max)
                nc.scalar.activation(E_sb[:, ic, :], negt[:, ic, :], AF.Exp,
                                     accum_out=rs[:, ic:ic + 1])
            # normalize in a second pass: keeps ic+1's select chain from
            # queueing behind ic's exp on the DVE FIFO
            for ic in range(EC):
                nc.vector.reciprocal(rsr[:, ic:ic + 1], rs[:, ic:ic + 1])
                nc.vector.tensor_scalar(E_sb[:, ic, :], E_sb[:, ic, :],
                                        rsr[:, ic:ic + 1], None, OP.mult)
            st["E_sb"] = E_sb

        def phase2c(n, st):
            """out = E^T @ H (emitted after the next phase1 so its H matmuls
            can fill the PE gap while this graph's softmax finishes)"""
            E_sb = st["E_sb"]; H_sb = st["H_sb"]
            for jc in range(EC):
                pO = ps_big.tile([P, D], F32, tag="big")
                for ic in range(EC):
                    nc.tensor.matmul(pO[:], E_sb[:, ic, jc * P:(jc + 1) * P],
                                     H_sb[:, ic, :],
                                     start=(ic == 0), stop=(ic == EC - 1))
                o_sb = small.tile([P, D], BF16, tag="osb")
                nc.scalar.copy(o_sb[:], pO[:])
                nc.sync.dma_start(out[n, jc * P:(jc + 1) * P, :], o_sb[:])

        def body_all(_iv=None):
            sts = {}
            run_prep_front()
            sts[0] = phase1(0)
            sts[1] = phase1(1)
            run_prep()
            phase2a(0, sts[0])
            for n in range(NG):
                if n + 1 < NG:
                    phase2a(n + 1, sts[n + 1])
                st = sts.pop(n)
                phase2b(n, st)
                phase2c(n, st)
                if n + 2 < NG:
                    sts[n + 2] = phase1(n + 2)

        if reps == 1:
            body_all()
        else:
            with tc.For_i(0, reps, 1) as _iv:
                body_all(_iv)
    return nc


_NC_CACHE = {}
TRACE = False
_LAST = {}


def _get_nc():
    if "nc" not in _NC_CACHE:
        nc = bacc.Bacc("TRN2", target_bir_lowering=False, debug=False)
        build(nc)
        nc.compile()
        _NC_CACHE["nc"] = nc
    return _NC_CACHE["nc"]


def kernel(input_state, adj, entity_mask, query_vec, W_type, a_type,
           qattn_W1, qattn_W2):
    import ml_dtypes
    from concourse import bass_utils
    bf16 = ml_dtypes.bfloat16
    nc = _get_nc()

    x_bf = np.ascontiguousarray(input_state, dtype=np.float32).astype(bf16)
    adj_i8 = np.ascontiguousarray(adj).astype(np.int8)
    qv = np.ascontiguousarray(query_vec, dtype=np.float32).astype(bf16)
    # aT[p, dc2, t] = a_type[t, dc2*128 + p]
    aT = np.ascontiguousarray(
        np.transpose(np.asarray(a_type, np.float32).reshape(NT, DC2, P),
                     (2, 1, 0)))
    WtT = np.ascontiguousarray(
        np.transpose(np.asarray(W_type, np.float32), (0, 2, 1))).astype(bf16)
    Wt2 = np.ascontiguousarray(np.asarray(W_type, np.float32)[2]).astype(bf16)
    W1_bf = np.ascontiguousarray(qattn_W1, dtype=np.float32).astype(bf16)
    W2_bf = np.ascontiguousarray(qattn_W2, dtype=np.float32).astype(bf16)

    in_maps = []
    for c in range(N_CORES):
        sl = slice(c * NG, (c + 1) * NG)
        # qT[p, kc, n] = qv[n, kc*128 + p]
        qT = np.ascontiguousarray(
            np.transpose(qv[sl].reshape(NG, KC, P), (2, 1, 0)))
        in_maps.append({
            "x": x_bf[sl], "adj": adj_i8[sl], "qT": qT, "aT": aT,
            "WtT": WtT, "Wt2": Wt2, "W1": W1_bf, "W2q": W2_bf,
        })
    res = bass_utils.run_bass_kernel_spmd(nc, in_maps, core_ids=list(range(N_CORES)),
                                          trace=TRACE, stitch_traces=TRACE)
    _LAST["exec_ns"] = res.exec_time_ns
    _LAST["mean_ns"] = res.mean_exec_time_ns
    _LAST["trace"] = res.instructions_and_trace
    _LAST["scope_times"] = res.per_core_scope_times
    out = np.concatenate([np.asarray(r["out"], np.float32) for r in res.results],
                         axis=0)
    return out
